# revision 6
# baseline (speedup 1.0000x reference)
"""Multi-head attention (shared QKV projection, floor-div scores) on 8 NeuronCores.

Problem: B=2, S=2048, HID=1024, NH=16, HD=64
    q = k = v = x @ Wq + bq          (reshaped to heads)
    scores = floor(q k^T / sqrt(64)) ; attn = softmax(scores)
    out = (attn v) @ Wo + bo

Sharding: core c handles batch c//4 and 4 heads ((c%4)*4 ..+4). Each core
computes its heads' contribution to out[b] = attn_out @ Wo; the host sums the
4 partials per batch and adds bo.

Single-pass fp16 device algorithm per core (rel err ~8e-3 vs the 2e-2 gate):
  - host pre-scales xT by 8^-0.5 (and bq by 8^-0.5, Wo by 8^0.5) so the PE
    score matmuls directly produce s8 = scores/sqrt(HD).
  - qT per head stored [65, S] fp16: rows 0-63 = q16, row 64 = 7.96875 const
    whose square (63.5009) rides the score contraction (K=65), implementing
    the +63.5 part of the floor-by-RNE trick for free.
  - v tiles = PE-transposed q16 slices (+ ones column for rowsum Z)
  - scores per (head, q-quarter J, k-tile i): [128,512] fp32 psum blocks.
    floor: n = RNE(s8 + 63.5 + 2^23), split across engines per unit of 2 i:
      * DVE units: tensor_single_scalar(+2^23) psum -> nt sbuf
      * ACT units: Identity activation (+2^23 bias) psum -> nt sbuf
      * PE "preset" units: K=1 const matmul writes 2^23 into psum first,
        score matmul accumulates; exp then reads psum directly (no round op)
    P = exp(n - (2^23+64+PSHIFT)) fp16 on ACT -> oT[65,512] += v_i^T @ P_i
  - Z normalization: rz = reciprocal_approx_fast(Z) on DVE, broadcast via
    DRAM round-trip DMA, oTn = oT * rz fp16 on GPSIMD
  - partial = oTn_pair^T @ Wo_pair fp16 -> fp16 partial out
"""

import math
import sys

sys.path.insert(0, "/opt/trn_rl_repo")

import numpy as np
import concourse.bass as bass
import concourse.bacc as bacc
import concourse.tile as tile
from concourse import mybir
from concourse.bass_utils import run_bass_kernel_spmd

F32 = mybir.dt.float32
F16 = mybir.dt.float16
ADD = mybir.AluOpType.add
MULT = mybir.AluOpType.mult
AF = mybir.ActivationFunctionType

B, S, HID, NH, HD = 2, 2048, 1024, 16, 64
HPC = 4          # heads per core
NCORES = 8
KT = HID // 128  # 8 k-tiles
QT = S // 128    # 16 q/s tiles
C23 = float(2 ** 23)
PSHIFT = 10.0    # P = e^(s_int - PSHIFT); cancels in softmax; keeps P < fp16 max
CROW = 7.96875   # CROW^2 = 63.50098 ~ 63.5 (floor offset, in-contraction)
SQ8 = 1.0 / math.sqrt(8.0)

# Per 8-unit (h, J) inner loop: which units use which floor-round engine.
# u == 7 -> PE preset (2^23 pre-written to psum by K=1 matmul)
# u == 3 -> ACT Identity round (only for odd heads; even heads use DVE)
PRESET_U = set()
ACT_U = set()

_NC_CACHE = None


def _build():
    nc = bacc.Bacc("TRN2", target_bir_lowering=False, debug=False,
                   num_devices=NCORES)

    x16d = nc.dram_tensor("x16", [HID, S], F16, kind="ExternalInput")
    wqd, wod, bqrd = [], [], []
    for p in range(2):
        wqd.append(nc.dram_tensor(f"wq{p}", [128, 1024], F16,
                                  kind="ExternalInput"))
        wod.append(nc.dram_tensor(f"wo{p}", [128, 1024], F16,
                                  kind="ExternalInput"))
        bqrd.append(nc.dram_tensor(f"bqr{p}", [1, 128], F16,
                                   kind="ExternalInput"))
    ident = nc.dram_tensor("ident", [128, 64], F16, kind="ExternalInput")
    part = nc.dram_tensor("part", [S, HID], F16, kind="ExternalOutput")
    rzscr = nc.dram_tensor("rzscr", [HPC, S], F32, kind="ExternalOutput")

    with tile.TileContext(nc) as tc:
        with (
            tc.tile_pool(name="cst", bufs=1) as cst,
            tc.tile_pool(name="big", bufs=1) as big,
            tc.tile_pool(name="ppool", bufs=3) as ppool,
            tc.tile_pool(name="ntpool", bufs=2) as ntpool,
            tc.tile_pool(name="osb", bufs=2) as osb,
            tc.tile_pool(name="zs", bufs=2) as zs,
            tc.tile_pool(name="otp", bufs=2) as otp,
            tc.tile_pool(name="ps_np", bufs=2, space="PSUM") as ps_np,
            tc.tile_pool(name="ps_ot", bufs=2, space="PSUM") as ps_ot,
            tc.tile_pool(name="ps_pq", bufs=1, space="PSUM") as ps_pq,
        ):
            # ---- constants / inputs ----
            b_negc = cst.tile([128, 1], F32, tag="b_negc")
            nc.vector.memset(b_negc[:], -(C23 + 64.0 + PSHIFT))
            onesr = cst.tile([1, S], F16, tag="onesr")
            nc.vector.memset(onesr[:], 1.0)
            c4096 = cst.tile([1, 128], F16, tag="c4096")
            nc.vector.memset(c4096[:], 4096.0)
            crows = cst.tile([1, S], F16, tag="crows")
            nc.vector.memset(crows[:], 2048.0)
            id_t = cst.tile([128, 64], F16, tag="id_t")
            nc.sync.dma_start(id_t[:], ident[:])
            wq_t, wo_t, bqr_t = [], [], []
            for p in range(2):
                w = cst.tile([128, 1024], F16, tag=f"wq_t{p}")
                nc.sync.dma_start(w[:], wqd[p][:])
                wq_t.append(w)
                w = cst.tile([128, 1024], F16, tag=f"wo_t{p}")
                nc.sync.dma_start(w[:], wod[p][:])
                wo_t.append(w)
                w = cst.tile([1, 128], F16, tag=f"bqr_t{p}")
                nc.sync.dma_start(w[:], bqrd[p][:])
                bqr_t.append(w)
            x16_t = []
            for t in range(KT):
                w = big.tile([128, S], F16, tag=f"x16_{t}")
                nc.sync.dma_start(w[:], x16d[t * 128:(t + 1) * 128, :])
                x16_t.append(w)

            # qT per head: rows 0-63 = q16, row 64 = CROW const
            q16_t = []
            for h in range(HPC):
                w = big.tile([65, S], F16, tag=f"q16_{h}")
                nc.vector.memset(w[64:65, :], CROW)
                q16_t.append(w)

            # ---- phase 1: qT projection (1-pass fp16) ----
            def emit_proj(p):
                for jh in range(2):
                    o = jh * 1024
                    pq = ps_pq.tile([128, 1024], F32, tag="pqps",
                                    name=f"pq{p}_{jh}")
                    for t in range(KT):
                        for c in range(2):
                            nc.tensor.matmul(
                                pq[:, c * 512:(c + 1) * 512],
                                wq_t[p][:, t * 128:(t + 1) * 128],
                                x16_t[t][:, o + c * 512:o + (c + 1) * 512],
                                start=(t == 0), stop=False,
                            )
                    for c in range(2):
                        nc.tensor.matmul(
                            pq[:, c * 512:(c + 1) * 512],
                            bqr_t[p][:],
                            onesr[:, o + c * 512:o + (c + 1) * 512],
                            start=False, stop=(c == 1),
                        )
                    for hx in range(2):
                        nc.vector.tensor_copy(
                            q16_t[2 * p + hx][0:64, o:o + 1024],
                            pq[hx * 64:hx * 64 + 64, :],
                        )

            # ---- phase 2: v tiles (transposed q16 + ones col) ----
            v_t = [None] * HPC

            def emit_v(h):
                vt = big.tile([128, QT * 65], F16, tag=f"v{h}")
                for half in range(2):
                    pv = ps_pq.tile([128, 512], F16, tag="pqps",
                                    name=f"pv{h}_{half}")
                    for tt in range(8):
                        i = half * 8 + tt
                        nc.tensor.transpose(
                            pv[:, tt * 64:(tt + 1) * 64],
                            q16_t[h][0:64, i * 128:(i + 1) * 128],
                            id_t[0:64, :],
                        )
                    dst = (vt[:, half * 520:half * 520 + 520]
                           .rearrange("p (t e) -> p t e", e=65)[:, :, 0:64])
                    src = pv[:, 0:512].rearrange("p (t e) -> p t e", e=64)
                    nc.vector.tensor_copy(dst, src)
                ones = vt[:].rearrange("p (t e) -> p t e", e=65)[:, :, 64:65]
                nc.vector.memset(ones, 1.0)
                v_t[h] = vt

            # ---- phase 3: per head, per q-quarter ----
            oTn_t = []
            for p in range(2):
                w = big.tile([128, S], F16, tag=f"oTn{p}")
                oTn_t.append(w)

            def emit_head(h):
                p, hx = h // 2, h % 2
                q16 = q16_t[h]
                oTsb = osb.tile([65, S], F32, tag="oTsb", name=f"oTsb{h}")
                for J in range(4):
                    jo = J * 512
                    poT = ps_ot.tile([65, 512], F32, tag="poTps",
                                     name=f"poT{h}_{J}")
                    for u in range(8):
                        i0 = 2 * u
                        preset = u in PRESET_U
                        act_rnd = (u in ACT_U) and (h % 2 == 1)
                        nP = ps_np.tile([128, 1024], F32, tag="nPps",
                                        name=f"nP{h}_{J}_{u}")
                        for half in range(2):
                            i = i0 + half
                            dst = nP[:, half * 512:(half + 1) * 512]
                            if preset:
                                nc.tensor.matmul(
                                    dst, c4096[:],
                                    crows[:, jo:jo + 512],
                                    start=True, stop=False,
                                )
                            nc.tensor.matmul(
                                dst,
                                q16[0:65, i * 128:(i + 1) * 128],
                                q16[0:65, jo:jo + 512],
                                start=not preset, stop=True,
                            )
                        pt = ppool.tile([128, 1024], F16, tag="ptile",
                                        name=f"pt{h}_{J}_{u}")
                        if preset:
                            nc.scalar.activation(pt[:], nP[:], AF.Exp,
                                                 bias=b_negc[:], scale=1.0)
                        else:
                            nt = ntpool.tile([128, 1024], F32, tag="ntile",
                                             name=f"nt{h}_{J}_{u}")
                            if act_rnd:
                                nc.scalar.activation(nt[:], nP[:],
                                                     AF.Identity,
                                                     bias=C23, scale=1.0)
                            else:
                                nc.vector.tensor_single_scalar(
                                    nt[:], nP[:], C23, ADD)
                            nc.scalar.activation(pt[:], nt[:], AF.Exp,
                                                 bias=b_negc[:], scale=1.0)
                        for half in range(2):
                            i = i0 + half
                            nc.tensor.matmul(
                                poT[:],
                                v_t[h][:, i * 65:(i + 1) * 65],
                                pt[:, half * 512:(half + 1) * 512],
                                start=(i == 0), stop=(i == QT - 1),
                            )
                    nc.vector.tensor_copy(oTsb[:, jo:jo + 512], poT[:])

                # normalization: rz = 1/Z; DRAM round-trip broadcast; GPSIMD
                rz = zs.tile([1, S], F32, tag="rz", name=f"rz{h}")
                nc.vector.reciprocal(rz[:], oTsb[64:65, :])
                nc.sync.dma_start(rzscr[h:h + 1, :], rz[:])
                repz = zs.tile([64, S], F32, tag="repz", name=f"repz{h}")
                nc.sync.dma_start(
                    repz[:], rzscr[h:h + 1, :].broadcast_to([64, S]))
                r = hx * 64
                nc.gpsimd.tensor_tensor(oTn_t[p][r:r + 64, :],
                                        oTsb[0:64, :], repz[:], MULT)

            for p in range(2):
                emit_proj(p)
                emit_v(2 * p)
                emit_v(2 * p + 1)
                emit_head(2 * p)
                emit_head(2 * p + 1)

            # ---- phase 4: output projection (fp16) ----
            for m in range(QT):
                po = ps_np.tile([128, 1024], F32, tag="nPps", name=f"po{m}")
                for c in range(2):
                    nc.tensor.matmul(
                        po[:, c * 512:(c + 1) * 512],
                        oTn_t[0][:, m * 128:(m + 1) * 128],
                        wo_t[0][:, c * 512:(c + 1) * 512],
                        start=True, stop=False,
                    )
                    nc.tensor.matmul(
                        po[:, c * 512:(c + 1) * 512],
                        oTn_t[1][:, m * 128:(m + 1) * 128],
                        wo_t[1][:, c * 512:(c + 1) * 512],
                        start=False, stop=True,
                    )
                ot = otp.tile([128, 1024], F16, tag="otile", name=f"ot{m}")
                if m % 2 == 0:
                    nc.vector.tensor_copy(ot[:], po[:])
                else:
                    nc.scalar.copy(ot[:], po[:])
                nc.sync.dma_start(part[m * 128:(m + 1) * 128, :], ot[:])

    nc.finalize()
    return nc


def _get_nc():
    global _NC_CACHE
    if _NC_CACHE is None:
        _NC_CACHE = _build()
    return _NC_CACHE


def make_in_maps(x, Wq, bq, Wo):
    eye = np.eye(64, dtype=np.float16)
    ident = np.vstack([eye, eye])
    in_maps = []
    for c in range(NCORES):
        b, hb = c // 4, (c % 4) * HPC
        xts = np.ascontiguousarray(x[b].T) * np.float32(SQ8)   # [1024, 2048]
        x16 = xts.astype(np.float16)
        m = {"x16": x16, "ident": ident}
        for p in range(2):
            lo = (hb + 2 * p) * HD          # first col/row of this head pair
            wq_cols = Wq[:, lo:lo + 128]    # [1024, 128]
            # lhsT k-tile layout: [128 part, 8 ktiles x 128]
            wqp = np.ascontiguousarray(
                wq_cols.reshape(KT, 128, 128).transpose(1, 0, 2).reshape(128, 1024)
            )
            m[f"wq{p}"] = wqp.astype(np.float16)
            m[f"wo{p}"] = (np.ascontiguousarray(Wo[lo:lo + 128, :])
                           * np.float32(1.0 / SQ8)).astype(np.float16)
            m[f"bqr{p}"] = (bq[None, lo:lo + 128]
                            * np.float32(SQ8)).astype(np.float16)
        in_maps.append(m)
    return in_maps


def kernel(x, Wq, bq, Wo, bo):
    x = np.asarray(x, np.float32)
    Wq = np.asarray(Wq, np.float32)
    bq = np.asarray(bq, np.float32)
    Wo = np.asarray(Wo, np.float32)
    bo = np.asarray(bo, np.float32)

    in_maps = make_in_maps(x, Wq, bq, Wo)
    res = run_bass_kernel_spmd(_get_nc(), in_maps, list(range(NCORES)))
    parts = [r["part"] for r in res.results]
    out = np.empty((B, S, HID), np.float32)
    for b in range(B):
        out[b] = (parts[4 * b].astype(np.float32)
                  + parts[4 * b + 1].astype(np.float32)
                  + parts[4 * b + 2].astype(np.float32)
                  + parts[4 * b + 3].astype(np.float32))
        out[b] += bo[None, :]
    return out


# revision 10
# speedup vs baseline: 1.0574x; 1.0574x over previous
"""Multi-head attention (shared QKV projection, floor-div scores) on 8 NeuronCores.

Problem: B=2, S=2048, HID=1024, NH=16, HD=64
    q = k = v = x @ Wq + bq          (reshaped to heads)
    scores = floor(q k^T / sqrt(64)) ; attn = softmax(scores)
    out = (attn v) @ Wo + bo

Sharding: core c handles batch c//4 and 4 heads ((c%4)*4 ..+4). Each core
computes its heads' contribution to out[b] = attn_out @ Wo; the host sums the
4 partials per batch and adds bo.

Single-pass fp16 device algorithm per core (rel err ~8e-3 vs the 2e-2 gate):
  - host pre-scales xT by 8^-0.5 (and bq by 8^-0.5, Wo by 8^0.5) so the PE
    score matmuls directly produce s8 = scores/sqrt(HD).
  - qT per head stored [65, S] fp16: rows 0-63 = q16, row 64 = 7.96875 const
    whose square (63.5009) rides the score contraction (K=65), implementing
    the +63.5 part of the floor-by-RNE trick for free.
  - v tiles = PE-transposed q16 slices (+ ones column for rowsum Z)
  - scores per (head, q-quarter J, k-tile i): [128,512] fp32 psum blocks.
    floor: n = RNE(s8 + 63.5 + 2^23), split across engines per unit of 2 i:
      * DVE units: tensor_single_scalar(+2^23) psum -> nt sbuf
      * ACT units: Identity activation (+2^23 bias) psum -> nt sbuf
      * PE "preset" units: K=1 const matmul writes 2^23 into psum first,
        score matmul accumulates; exp then reads psum directly (no round op)
    P = exp(n - (2^23+64+PSHIFT)) fp16 on ACT -> oT[65,512] += v_i^T @ P_i
  - Z normalization: rz = reciprocal_approx_fast(Z) on DVE, broadcast via
    DRAM round-trip DMA, oTn = oT * rz fp16 on GPSIMD
  - partial = oTn_pair^T @ Wo_pair fp16 -> fp16 partial out
"""

import math
import sys

sys.path.insert(0, "/opt/trn_rl_repo")

import numpy as np
import concourse.bass as bass
import concourse.bacc as bacc
import concourse.tile as tile
from concourse import mybir
from concourse.bass_utils import run_bass_kernel_spmd

F32 = mybir.dt.float32
F16 = mybir.dt.float16
ADD = mybir.AluOpType.add
MULT = mybir.AluOpType.mult
AF = mybir.ActivationFunctionType

B, S, HID, NH, HD = 2, 2048, 1024, 16, 64
HPC = 4          # heads per core
NCORES = 8
KT = HID // 128  # 8 k-tiles
QT = S // 128    # 16 q/s tiles
C23 = float(2 ** 23)
PSHIFT = 10.0    # P = e^(s_int - PSHIFT); cancels in softmax; keeps P < fp16 max
CROW = 7.96875   # CROW^2 = 63.50098 ~ 63.5 (floor offset, in-contraction)
SQ8 = 1.0 / math.sqrt(8.0)

# Per 8-unit (h, J) inner loop: which units use which floor-round engine.
# u == 7 -> PE preset (2^23 pre-written to psum by K=1 matmul)
# u == 3 -> ACT Identity round (only for odd heads; even heads use DVE)
PRESET_U = set()
ACT_U = set()

_NC_CACHE = None


def _build():
    nc = bacc.Bacc("TRN2", target_bir_lowering=False, debug=False,
                   num_devices=NCORES)

    x16d = nc.dram_tensor("x16", [HID, S], F16, kind="ExternalInput")
    wqd, wod, bqrd = [], [], []
    for p in range(2):
        wqd.append(nc.dram_tensor(f"wq{p}", [128, 1024], F16,
                                  kind="ExternalInput"))
        wod.append(nc.dram_tensor(f"wo{p}", [128, 1024], F16,
                                  kind="ExternalInput"))
        bqrd.append(nc.dram_tensor(f"bqr{p}", [1, 128], F16,
                                   kind="ExternalInput"))
    ident = nc.dram_tensor("ident", [128, 64], F16, kind="ExternalInput")
    part = nc.dram_tensor("part", [S, HID], F16, kind="ExternalOutput")
    rzscr = nc.dram_tensor("rzscr", [HPC, S], F32, kind="ExternalOutput")

    with tile.TileContext(nc) as tc:
        with (
            tc.tile_pool(name="cst", bufs=1) as cst,
            tc.tile_pool(name="big", bufs=1) as big,
            tc.tile_pool(name="ppool", bufs=3) as ppool,
            tc.tile_pool(name="ntpool", bufs=2) as ntpool,
            tc.tile_pool(name="osb", bufs=2) as osb,
            tc.tile_pool(name="zs", bufs=2) as zs,
            tc.tile_pool(name="otp", bufs=2) as otp,
            tc.tile_pool(name="ps_np", bufs=2, space="PSUM") as ps_np,
            tc.tile_pool(name="ps_ot", bufs=2, space="PSUM") as ps_ot,
            tc.tile_pool(name="ps_pq", bufs=1, space="PSUM") as ps_pq,
        ):
            # ---- constants / inputs ----
            b_negc = cst.tile([128, 1], F32, tag="b_negc")
            nc.vector.memset(b_negc[:], -(C23 + 64.0 + PSHIFT))
            onesr = cst.tile([1, S], F16, tag="onesr")
            nc.vector.memset(onesr[:], 1.0)
            c4096 = cst.tile([1, 128], F16, tag="c4096")
            nc.vector.memset(c4096[:], 4096.0)
            crows = cst.tile([1, S], F16, tag="crows")
            nc.vector.memset(crows[:], 2048.0)
            id_t = cst.tile([128, 64], F16, tag="id_t")
            nc.sync.dma_start(id_t[:], ident[:])
            wq_t, wo_t, bqr_t = [], [], []
            for p in range(2):
                w = cst.tile([128, 1024], F16, tag=f"wq_t{p}")
                nc.sync.dma_start(w[:], wqd[p][:])
                wq_t.append(w)
                w = cst.tile([128, 1024], F16, tag=f"wo_t{p}")
                nc.sync.dma_start(w[:], wod[p][:])
                wo_t.append(w)
                w = cst.tile([1, 128], F16, tag=f"bqr_t{p}")
                nc.sync.dma_start(w[:], bqrd[p][:])
                bqr_t.append(w)
            x16_t = []
            for t in range(KT):
                w = big.tile([128, S], F16, tag=f"x16_{t}")
                nc.sync.dma_start(w[:], x16d[t * 128:(t + 1) * 128, :])
                x16_t.append(w)

            # qT per head: rows 0-63 = q16, row 64 = CROW const
            q16_t = []
            for h in range(HPC):
                w = big.tile([65, S], F16, tag=f"q16_{h}")
                nc.vector.memset(w[64:65, :], CROW)
                q16_t.append(w)

            # ---- phase 1: qT projection (1-pass fp16) ----
            def emit_proj(p):
                for jh in range(2):
                    o = jh * 1024
                    pq = ps_pq.tile([128, 1024], F32, tag="pqps",
                                    name=f"pq{p}_{jh}")
                    for t in range(KT):
                        for c in range(2):
                            nc.tensor.matmul(
                                pq[:, c * 512:(c + 1) * 512],
                                wq_t[p][:, t * 128:(t + 1) * 128],
                                x16_t[t][:, o + c * 512:o + (c + 1) * 512],
                                start=(t == 0), stop=False,
                            )
                    for c in range(2):
                        nc.tensor.matmul(
                            pq[:, c * 512:(c + 1) * 512],
                            bqr_t[p][:],
                            onesr[:, o + c * 512:o + (c + 1) * 512],
                            start=False, stop=(c == 1),
                        )
                    for hx in range(2):
                        nc.vector.tensor_copy(
                            q16_t[2 * p + hx][0:64, o:o + 1024],
                            pq[hx * 64:hx * 64 + 64, :],
                        )

            # ---- phase 2: v tiles (transposed q16 + ones col) ----
            v_t = [None] * HPC

            def emit_v(h):
                vt = big.tile([128, QT * 65], F16, tag=f"v{h}")
                for half in range(2):
                    pv = ps_pq.tile([128, 512], F16, tag="pqps",
                                    name=f"pv{h}_{half}")
                    for tt in range(8):
                        i = half * 8 + tt
                        nc.tensor.transpose(
                            pv[:, tt * 64:(tt + 1) * 64],
                            q16_t[h][0:64, i * 128:(i + 1) * 128],
                            id_t[0:64, :],
                        )
                    dst = (vt[:, half * 520:half * 520 + 520]
                           .rearrange("p (t e) -> p t e", e=65)[:, :, 0:64])
                    src = pv[:, 0:512].rearrange("p (t e) -> p t e", e=64)
                    nc.vector.tensor_copy(dst, src)
                ones = vt[:].rearrange("p (t e) -> p t e", e=65)[:, :, 64:65]
                nc.vector.memset(ones, 1.0)
                v_t[h] = vt

            # ---- phase 3: per head, per q-quarter ----
            oTn_t = []
            for p in range(2):
                w = big.tile([128, S], F16, tag=f"oTn{p}")
                oTn_t.append(w)

            def emit_pair_attn(p):
                """Software-pipelined J-loop over both heads of pair p.

                Score matmuls for unit k+1 are emitted before the
                round/exp/PV tail of unit k so the PE never sits behind the
                DVE->ACT chain in its own queue (head-of-line blocking).
                """
                heads = (2 * p, 2 * p + 1)
                oTsb = {h: osb.tile([65, S], F32, tag="oTsb",
                                    name=f"oTsb{h}") for h in heads}
                poT_t = {}

                def emit_tail(h, J, u, nP):
                    jo = J * 512
                    if u == 0:
                        poT_t[(h, J)] = ps_ot.tile(
                            [65, 512], F32, tag="poTps", name=f"poT{h}_{J}")
                    poT = poT_t[(h, J)]
                    pt = ppool.tile([128, 1024], F16, tag="ptile",
                                    name=f"pt{h}_{J}_{u}")
                    preset = u in PRESET_U
                    act_rnd = (u in ACT_U) and (h % 2 == 1)
                    if preset:
                        nc.scalar.activation(pt[:], nP[:], AF.Exp,
                                             bias=b_negc[:], scale=1.0)
                    else:
                        nt = ntpool.tile([128, 1024], F32, tag="ntile",
                                         name=f"nt{h}_{J}_{u}")
                        if act_rnd:
                            nc.scalar.activation(nt[:], nP[:], AF.Identity,
                                                 bias=C23, scale=1.0)
                        else:
                            nc.vector.tensor_single_scalar(
                                nt[:], nP[:], C23, ADD)
                        nc.scalar.activation(pt[:], nt[:], AF.Exp,
                                             bias=b_negc[:], scale=1.0)
                    for half in range(2):
                        i = 2 * u + half
                        nc.tensor.matmul(
                            poT[:],
                            v_t[h][:, i * 65:(i + 1) * 65],
                            pt[:, half * 512:(half + 1) * 512],
                            start=(i == 0), stop=(i == QT - 1),
                        )
                    if u == 7:
                        nc.vector.tensor_copy(oTsb[h][:, jo:jo + 512],
                                              poT[:])
                        if J == 3:
                            # rz = 1/Z (Z on partition 0 via ones-first v);
                            # DRAM round-trip broadcast; GPSIMD multiply
                            zrow = zs.tile([1, S], F32, tag="zrow",
                                           name=f"zrow{h}")
                            nc.gpsimd.tensor_copy(zrow[:], oTsb[h][64:65, :])
                            rz = zs.tile([1, S], F32, tag="rz",
                                         name=f"rz{h}")
                            nc.vector.reciprocal_approx_fast(
                                rz[:], zrow[:])
                            nc.sync.dma_start(rzscr[h:h + 1, :], rz[:])
                            repz = zs.tile([64, S], F32, tag="repz",
                                           name=f"repz{h}")
                            nc.sync.dma_start(
                                repz[:],
                                rzscr[h:h + 1, :].broadcast_to([64, S]))
                            r = (h % 2) * 64
                            nc.gpsimd.tensor_tensor(
                                oTn_t[p][r:r + 64, :],
                                oTsb[h][0:64, :], repz[:], MULT)

                pending = None
                for h in heads:
                    q16 = q16_t[h]
                    for J in range(4):
                        jo = J * 512
                        for u in range(8):
                            preset = u in PRESET_U
                            nP = ps_np.tile([128, 1024], F32, tag="nPps",
                                            name=f"nP{h}_{J}_{u}")
                            for half in range(2):
                                i = 2 * u + half
                                dst = nP[:, half * 512:(half + 1) * 512]
                                if preset:
                                    nc.tensor.matmul(
                                        dst, c4096[:],
                                        crows[:, jo:jo + 512],
                                        start=True, stop=False,
                                    )
                                nc.tensor.matmul(
                                    dst,
                                    q16[0:65, i * 128:(i + 1) * 128],
                                    q16[0:65, jo:jo + 512],
                                    start=not preset, stop=True,
                                )
                            if pending is not None:
                                emit_tail(*pending)
                            pending = (h, J, u, nP)
                emit_tail(*pending)

            for p in range(2):
                emit_proj(p)
                emit_v(2 * p)
                emit_v(2 * p + 1)
                emit_pair_attn(p)

            # ---- phase 4: output projection (fp16) ----
            for m in range(QT):
                po = ps_np.tile([128, 1024], F32, tag="nPps", name=f"po{m}")
                for c in range(2):
                    nc.tensor.matmul(
                        po[:, c * 512:(c + 1) * 512],
                        oTn_t[0][:, m * 128:(m + 1) * 128],
                        wo_t[0][:, c * 512:(c + 1) * 512],
                        start=True, stop=False,
                    )
                    nc.tensor.matmul(
                        po[:, c * 512:(c + 1) * 512],
                        oTn_t[1][:, m * 128:(m + 1) * 128],
                        wo_t[1][:, c * 512:(c + 1) * 512],
                        start=False, stop=True,
                    )
                ot = otp.tile([128, 1024], F16, tag="otile", name=f"ot{m}")
                if m % 2 == 0:
                    nc.vector.tensor_copy(ot[:], po[:])
                else:
                    nc.scalar.copy(ot[:], po[:])
                nc.sync.dma_start(part[m * 128:(m + 1) * 128, :], ot[:])

    nc.finalize()
    return nc


def _get_nc():
    global _NC_CACHE
    if _NC_CACHE is None:
        _NC_CACHE = _build()
    return _NC_CACHE


def make_in_maps(x, Wq, bq, Wo):
    eye = np.eye(64, dtype=np.float16)
    ident = np.vstack([eye, eye])
    in_maps = []
    for c in range(NCORES):
        b, hb = c // 4, (c % 4) * HPC
        xts = np.ascontiguousarray(x[b].T) * np.float32(SQ8)   # [1024, 2048]
        x16 = xts.astype(np.float16)
        m = {"x16": x16, "ident": ident}
        for p in range(2):
            lo = (hb + 2 * p) * HD          # first col/row of this head pair
            wq_cols = Wq[:, lo:lo + 128]    # [1024, 128]
            # lhsT k-tile layout: [128 part, 8 ktiles x 128]
            wqp = np.ascontiguousarray(
                wq_cols.reshape(KT, 128, 128).transpose(1, 0, 2).reshape(128, 1024)
            )
            m[f"wq{p}"] = wqp.astype(np.float16)
            m[f"wo{p}"] = (np.ascontiguousarray(Wo[lo:lo + 128, :])
                           * np.float32(1.0 / SQ8)).astype(np.float16)
            m[f"bqr{p}"] = (bq[None, lo:lo + 128]
                            * np.float32(SQ8)).astype(np.float16)
        in_maps.append(m)
    return in_maps


def kernel(x, Wq, bq, Wo, bo):
    x = np.asarray(x, np.float32)
    Wq = np.asarray(Wq, np.float32)
    bq = np.asarray(bq, np.float32)
    Wo = np.asarray(Wo, np.float32)
    bo = np.asarray(bo, np.float32)

    in_maps = make_in_maps(x, Wq, bq, Wo)
    res = run_bass_kernel_spmd(_get_nc(), in_maps, list(range(NCORES)))
    parts = [r["part"] for r in res.results]
    out = np.empty((B, S, HID), np.float32)
    for b in range(B):
        out[b] = (parts[4 * b].astype(np.float32)
                  + parts[4 * b + 1].astype(np.float32)
                  + parts[4 * b + 2].astype(np.float32)
                  + parts[4 * b + 3].astype(np.float32))
        out[b] += bo[None, :]
    return out
